# revision 2
# baseline (speedup 1.0000x reference)
"""Self-contained Trainium2 Bass kernel for nn_Model_16801912062040 (dense_cnn).

Sharding: batch-parallel, 2 samples per core across 8 cores. The dynamic conv
collapses algebraically: y[n,(m,o)] = alpha[m,o]*(x2[n] (x) Wi[o]) and the
conv_transpose contraction over (m,o) reduces to
    z[n,i] = sum_o s_o * (Y0[n,o] (x)_full flip(Wi[o,i])),  s_o = sum_m alpha[m,o]^2
so only the per-channel scalar s (128 floats) couples samples. Each core
computes pooled stats for its 2 samples; one AllGather per layer shares them;
the tiny alpha path is recomputed on every core.
"""
import math
import numpy as np

import concourse.bass as bass
import concourse.tile as tile
from concourse import bacc, mybir
from concourse.bass_utils import run_bass_kernel_spmd

N_CORES = 8
BPC = 2            # samples per core
B, L, CIN, D = 16, 192, 21, 128
P, S, NT, PRED, COUT = 24, 12, 16, 96, 21
LAYERS = 2
BN_EPS = 1e-5
F32 = mybir.dt.float32

_CACHE = {}
LAST_RESULT = None


def _pos_embed():
    pos = np.arange(L, dtype=np.float32)[:, None]
    div = np.exp(np.arange(0, D, 2, dtype=np.float32) * (-math.log(10000.0) / D))
    pe = np.zeros((L, D), np.float32)
    pe[:, 0::2] = np.sin(pos * div)
    pe[:, 1::2] = np.cos(pos * div)
    return pe


def _prep_consts(token_w, patch_w, patch_b, Wi, pconv_w, pconv_b, bn_g, bn_b,
                 aconv_w, fc1_w, fc1_b, fc2_w, fc2_b):
    c = {}
    c["tokA"] = np.ascontiguousarray(token_w.transpose(1, 2, 0))              # [c,k,d]
    c["tokR"] = np.ascontiguousarray(np.roll(token_w, -64, 0).transpose(1, 2, 0))
    posT = np.ascontiguousarray(_pos_embed().T)                               # [d,l]
    c["posT"] = posT
    c["posR"] = np.ascontiguousarray(np.roll(posT, -64, 0))
    # patch conv as dense banded matmul over m (xe column), j = nt*24 + p
    W2p = np.zeros((205, 384), np.float32)
    for nt in range(NT):
        for p_ in range(P):
            j = nt * 24 + p_
            for k in range(P):
                m = 12 * nt + k
                W2p[min(m, 191), j] += patch_w[p_, 0, k]   # replicate-pad fold
            W2p[204, j] = patch_b[p_]
    c["w2pa"] = np.ascontiguousarray(W2p[0:128])                              # [128,384]
    c["w2pb"] = np.ascontiguousarray(np.concatenate([W2p[128:192], W2p[204:205]], 0))  # [65,384]
    A = Wi.transpose(0, 3, 4, 2, 1).reshape(LAYERS, 9, D, D)                  # [l,tap,i,o]
    c["wiT"] = np.ascontiguousarray(A.transpose(2, 0, 1, 3).reshape(D, LAYERS * 9 * D))
    Wf = Wi[:, :, :, ::-1, ::-1]
    Bt = Wf.transpose(0, 3, 4, 1, 2).reshape(LAYERS, 9, D, D)                 # [l,tap,o,i]
    c["w2T"] = np.ascontiguousarray(Bt.transpose(2, 0, 1, 3).reshape(D, LAYERS * 9 * D))
    Ct = pconv_w.transpose(0, 3, 2, 1) / 24.0                                 # [l,k,c,o]
    c["pcvT"] = np.ascontiguousarray(Ct.transpose(2, 0, 1, 3).reshape(D, LAYERS * 3 * D))
    c["awT"] = np.ascontiguousarray((aconv_w.transpose(2, 0, 1) / 16.0).reshape(D, LAYERS * D))
    c["bns"] = np.ascontiguousarray((bn_g / np.sqrt(1.0 + BN_EPS)).T)         # [128,2]
    c["bnb"] = np.ascontiguousarray(bn_b.T)
    c["fc1e"] = np.ascontiguousarray(fc1_w.T.reshape(D, 3, PRED).reshape(D, 3 * PRED))
    c["fc1b"] = np.ascontiguousarray(fc1_b[None, :])                          # [1,96]
    c["fc2T"] = np.ascontiguousarray(fc2_w.T)                                 # [128,21]
    c["fc2b"] = np.ascontiguousarray(fc2_b[None, :])                          # [1,21]
    # pconv bias folded via BN: relu(x*scale + (pconv_b*scale + beta))
    c["bnb"] = np.ascontiguousarray((pconv_b * (bn_g / np.sqrt(1.0 + BN_EPS)) + bn_b).T)
    return c


def _build():
    nc = bacc.Bacc("TRN2", target_bir_lowering=False, debug=False, num_devices=N_CORES)

    def param(name, shape):
        return nc.declare_dram_parameter(name, list(shape), F32, isOutput=False)

    xtp = param("xtp", (BPC, CIN, 194))
    tokA = param("tokA", (CIN, 3, D)); tokR = param("tokR", (CIN, 3, D))
    posT = param("posT", (D, L)); posR = param("posR", (D, L))
    w2pa = param("w2pa", (D, 384)); w2pb = param("w2pb", (65, 384))
    wiT = param("wiT", (D, LAYERS * 9 * D)); w2T = param("w2T", (D, LAYERS * 9 * D))
    pcvT = param("pcvT", (D, LAYERS * 3 * D)); awT = param("awT", (D, LAYERS * D))
    bns = param("bns", (D, LAYERS)); bnb = param("bnb", (D, LAYERS))
    fc1e = param("fc1e", (D, 3 * PRED)); fc1b = param("fc1b", (1, PRED))
    fc2T = param("fc2T", (D, COUT)); fc2b = param("fc2b", (1, COUT))
    out = nc.declare_dram_parameter("out", [BPC, PRED, COUT], F32, isOutput=True)

    agin = [nc.dram_tensor(f"agin{l}", [D, BPC, NT], F32) for l in range(LAYERS)]
    agout = [nc.dram_tensor(f"agout{l}", [N_CORES, D, BPC, NT], F32, addr_space="Shared")
             for l in range(LAYERS)]

    RELU = mybir.ActivationFunctionType.Relu
    SQUARE = mybir.ActivationFunctionType.Square
    ADD = mybir.AluOpType.add
    AX = mybir.AxisListType.X

    with tile.TileContext(nc) as tc:
        with tc.tile_pool(name="w", bufs=1) as wp, \
             tc.tile_pool(name="act", bufs=2) as ap, \
             tc.tile_pool(name="x2p", bufs=6) as xp, \
             tc.tile_pool(name="ps", bufs=1, space="PSUM") as pp:

            def wload(handle, shape, tag):
                t = wp.tile(list(shape), F32, tag=tag)
                nc.sync.dma_start(out=t[:], in_=handle[tuple(slice(None) for _ in shape)])
                return t

            tokA_sb = wload(tokA, (CIN, 3, D), "tokA")
            tokR_sb = wload(tokR, (CIN, 3, D), "tokR")
            posT_sb = wload(posT, (D, L), "posT")
            posR_sb = wload(posR, (D, L), "posR")
            w2pa_sb = wload(w2pa, (D, 384), "w2pa")
            w2pb_sb = wload(w2pb, (65, 384), "w2pb")
            wiT_sb = wload(wiT, (D, LAYERS * 9 * D), "wiT")
            w2T_sb = wload(w2T, (D, LAYERS * 9 * D), "w2T")
            pcvT_sb = wload(pcvT, (D, LAYERS * 3 * D), "pcvT")
            awT_sb = wload(awT, (D, LAYERS * D), "awT")
            bns_sb = wload(bns, (D, LAYERS), "bns")
            bnb_sb = wload(bnb, (D, LAYERS), "bnb")
            fc1e_sb = wload(fc1e, (D, 3 * PRED), "fc1e")
            fc1b_sb = wload(fc1b, (1, PRED), "fc1b")
            fc2T_sb = wload(fc2T, (D, COUT), "fc2T")
            fc2b_sb = wload(fc2b, (1, COUT), "fc2b")
            ones_sb = wp.tile([1, D], F32, tag="ones")
            nc.vector.memset(ones_sb[:], 1.0)

            # ---------------- embedding ----------------
            x2 = []
            for n in range(BPC):
                xt_sb = ap.tile([CIN, 194], F32, tag="xt")
                nc.sync.dma_start(out=xt_sb[:], in_=xtp[n, :, :])
                embs = []
                for tok_sb, pos_sb in ((tokA_sb, posT_sb), (tokR_sb, posR_sb)):
                    e_ps = pp.tile([D, L], F32, tag="ps", bufs=6)
                    for k in range(3):
                        nc.tensor.matmul(e_ps[:], lhsT=tok_sb[:, k, :],
                                         rhs=xt_sb[:, k:k + L],
                                         start=(k == 0), stop=(k == 2))
                    e_sb = ap.tile([D, L], F32, tag="emb_sb", bufs=2)
                    nc.vector.tensor_add(out=e_sb[:], in0=e_ps[:], in1=pos_sb[:])
                    embs.append(e_sb)
                eT = embs[0][:].rearrange("p (s c) -> p s c", c=3)
                eR = embs[1][:].rearrange("p (s c) -> p s c", c=3)
                xeA = ap.tile([D, 64, 2], F32, tag="xeA")
                xeB = ap.tile([65, 64, 2], F32, tag="xeB")
                nc.vector.tensor_copy(out=xeA[:, :, 0], in_=eT[:, :, 0])
                nc.vector.tensor_copy(out=xeA[0:64, :, 1], in_=eR[0:64, :, 1])
                nc.vector.tensor_copy(out=xeA[64:128, :, 1], in_=eR[64:128, :, 2])
                nc.scalar.copy(out=xeB[0:64, :, 0], in_=eT[0:64, :, 1])
                nc.scalar.copy(out=xeB[0:64, :, 1], in_=eR[0:64, :, 2])
                nc.vector.memset(xeB[64:65, :, :], 1.0)
                pcs = []
                for e in range(3):
                    pc_ps = pp.tile([D, NT, 8], F32, tag="ps", bufs=6)
                    nc.tensor.matmul(pc_ps[:], lhsT=w2pa_sb[:, 128 * e:128 * (e + 1)],
                                     rhs=xeA[:], start=True, stop=False)
                    nc.tensor.matmul(pc_ps[:], lhsT=w2pb_sb[:, 128 * e:128 * (e + 1)],
                                     rhs=xeB[:], start=False, stop=True)
                    pcs.append(pc_ps)
                x2n = xp.tile([D, NT, 24], F32, tag="x2")
                for u in range(2):
                    for w_ in range(12):
                        i2 = 2 * w_ + u
                        q, e = divmod(i2, 3)
                        eng = nc.vector.tensor_copy if i2 % 2 == 0 else (
                            lambda out, in_: nc.scalar.copy(out=out, in_=in_))
                        eng(out=x2n[:, :, u * 12 + w_], in_=pcs[e][:, :, q])
                x2.append(x2n)

            # ---------------- TimesBlocks ----------------
            for l in range(LAYERS):
                pp_loc = ap.tile([D, BPC, NT], F32, tag="pp_loc")
                for n in range(BPC):
                    nc.vector.tensor_reduce(out=pp_loc[:, n, :], in_=x2[n][:],
                                            axis=AX, op=ADD)
                nc.sync.dma_start(out=agin[l][:, :, :], in_=pp_loc[:])
                nc.gpsimd.collective_compute(
                    "AllGather", mybir.AluOpType.bypass,
                    replica_groups=[list(range(N_CORES))],
                    ins=[agin[l][:, :, :]], outs=[agout[l][:, :, :, :]])
                pp_all = ap.tile([D, N_CORES, BPC, NT], F32, tag="pp_all")
                nc.sync.dma_start(
                    out=pp_all[:],
                    in_=agout[l][:, :, :, :].rearrange("r c s t -> c r s t"))
                ppv = pp_all[:].rearrange("c r s t -> c (r s) t")

                # Y0 convs (independent of alpha path -> overlap the collective)
                y0ps = []
                for n in range(BPC):
                    y0_ps = pp.tile([D, 14, 22], F32, tag="ps", bufs=6)
                    for dp in range(3):
                        for dq in range(3):
                            tap = 3 * dp + dq
                            o = (l * 9 + tap) * D
                            nc.tensor.matmul(y0_ps[:], lhsT=wiT_sb[:, o:o + D],
                                             rhs=x2[n][:, dp:dp + 14, dq:dq + 22],
                                             start=(tap == 0), stop=(tap == 8))
                    y0ps.append(y0_ps)

                # alpha path (all 16 samples, redundant on every core)
                ppc_ps = pp.tile([D, B, NT], F32, tag="ps", bufs=6)
                for k in range(3):
                    o = (l * 3 + k) * D
                    if k == 0:
                        nc.tensor.matmul(ppc_ps[:, :, 1:NT], lhsT=pcvT_sb[:, o:o + D],
                                         rhs=ppv[:, :, 0:NT - 1], start=True, stop=False)
                    elif k == 1:
                        nc.tensor.matmul(ppc_ps[:], lhsT=pcvT_sb[:, o:o + D],
                                         rhs=ppv[:], start=False, stop=False)
                    else:
                        nc.tensor.matmul(ppc_ps[:, :, 0:NT - 1], lhsT=pcvT_sb[:, o:o + D],
                                         rhs=ppv[:, :, 1:NT], start=False, stop=True)
                ppc_sb = ap.tile([D, B, NT], F32, tag="ppc_sb")
                nc.scalar.activation(out=ppc_sb[:], in_=ppc_ps[:], func=RELU,
                                     bias=bnb_sb[:, l:l + 1], scale=bns_sb[:, l:l + 1])
                pooled = ap.tile([D, B], F32, tag="pooled")
                nc.vector.tensor_reduce(out=pooled[:], in_=ppc_sb[:], axis=AX, op=ADD)
                al_ps = pp.tile([D, B], F32, tag="ps", bufs=6)
                nc.tensor.matmul(al_ps[:], lhsT=awT_sb[:, l * D:(l + 1) * D],
                                 rhs=pooled[:], start=True, stop=True)
                asq = ap.tile([D, B], F32, tag="asq")
                nc.scalar.activation(out=asq[:], in_=al_ps[:], func=SQUARE,
                                     bias=1.0, scale=1.0)
                s_sb = ap.tile([D, 1], F32, tag="s")
                nc.vector.tensor_reduce(out=s_sb[:], in_=asq[:], axis=AX, op=ADD)

                for n in range(BPC):
                    y0p = ap.tile([D, 18, 26], F32, tag="y0p", bufs=2)
                    nc.gpsimd.memset(y0p[:], 0.0)
                    nc.vector.tensor_scalar_mul(out=y0p[:, 2:16, 2:24],
                                                in0=y0ps[n][:], scalar1=s_sb[:])
                    z_ps = pp.tile([D, NT, 24], F32, tag="ps", bufs=6)
                    for dp in range(3):
                        for dq in range(3):
                            tap = 3 * dp + dq
                            o = (l * 9 + tap) * D
                            nc.tensor.matmul(z_ps[:], lhsT=w2T_sb[:, o:o + D],
                                             rhs=y0p[:, dp:dp + 16, dq:dq + 24],
                                             start=(tap == 0), stop=(tap == 8))
                    x2n = xp.tile([D, NT, 24], F32, tag="x2")
                    nc.vector.tensor_add(out=x2n[:], in0=z_ps[:], in1=x2[n][:])
                    x2[n] = x2n

            # ---------------- heads ----------------
            for n in range(BPC):
                x2f = x2[n][:].rearrange("p a b -> p (a b)")
                y1_ps = pp.tile([D, PRED], F32, tag="ps", bufs=6)
                for e in range(3):
                    nc.tensor.matmul(y1_ps[:], lhsT=x2f[:, 128 * e:128 * (e + 1)],
                                     rhs=fc1e_sb[:, PRED * e:PRED * (e + 1)],
                                     start=(e == 0), stop=False)
                nc.tensor.matmul(y1_ps[:], lhsT=ones_sb[:], rhs=fc1b_sb[:],
                                 start=False, stop=True)
                y1_sb = ap.tile([D, PRED], F32, tag="y1sb")
                nc.scalar.copy(out=y1_sb[:], in_=y1_ps[:])
                o_ps = pp.tile([PRED, COUT], F32, tag="ps", bufs=6)
                nc.tensor.matmul(o_ps[:], lhsT=y1_sb[:], rhs=fc2T_sb[:],
                                 start=True, stop=False)
                nc.tensor.matmul(o_ps[:], lhsT=ones_sb[:, 0:PRED], rhs=fc2b_sb[:],
                                 start=False, stop=True)
                o_sb = ap.tile([PRED, COUT], F32, tag="osb")
                nc.vector.tensor_copy(out=o_sb[:], in_=o_ps[:])
                nc.sync.dma_start(out=out[n, :, :], in_=o_sb[:])

    nc.finalize()
    return nc


def kernel(**inputs):
    global LAST_RESULT
    inputs = {k: np.ascontiguousarray(np.asarray(v, np.float32)) for k, v in inputs.items()}
    if "nc" not in _CACHE:
        _CACHE["nc"] = _build()
    nc = _CACHE["nc"]
    c = _prep_consts(
        inputs["token_w"], inputs["patch_w"], inputs["patch_b"], inputs["Wi"],
        inputs["pconv_w"], inputs["pconv_b"], inputs["bn_g"], inputs["bn_b"],
        inputs["aconv_w"], inputs["fc1_w"], inputs["fc1_b"], inputs["fc2_w"],
        inputs["fc2_b"])
    xtp_full = np.pad(inputs["x"].transpose(0, 2, 1), ((0, 0), (0, 0), (1, 1)),
                      mode="wrap").astype(np.float32)
    in_maps = []
    for core in range(N_CORES):
        m = dict(c)
        m["xtp"] = np.ascontiguousarray(xtp_full[BPC * core:BPC * (core + 1)])
        in_maps.append(m)
    import os
    res = run_bass_kernel_spmd(nc, in_maps, core_ids=list(range(N_CORES)),
                               trace=bool(os.environ.get("BASS_TRACE")))
    LAST_RESULT = res
    return np.concatenate([res.results[cid]["out"] for cid in range(N_CORES)], axis=0)
